# revision 5
# baseline (speedup 1.0000x reference)
"""AttentionRNN Trainium2 kernel.

Strategy (memory-regime): the output projection logits = out @ fc_W.T + fc_b
is 99.6% of the FLOPs and ~all of the memory traffic (524 MB of f32 logits).
It runs on the 8 NeuronCores, sharded over the vocab dimension (each core
reads its fc_W slice [4000, 256] and writes logits [32*128, 4000]) with
float32r matmuls (measured 1.5e-4 rel err on HW, 4x the fp32 PE rate).
The tiny strictly-sequential attention-RNN recurrence (0.13 GFLOP over 128
dependent steps) runs in exact fp32 on host.
"""
import numpy as np

import concourse.bass as bass
import concourse.mybir as mybir
import concourse.tile as tile
from concourse import bass_utils

B, T, V, E, H = 32, 128, 32000, 128, 256
NEG = -1e9
N_CORES = 8
VLOC = V // N_CORES            # 4000
NT = 8                         # V tiles per core
NN = VLOC // NT                # 500 columns per tile
M = B * T                      # 4096 rows
MT = M // 128                  # 32 row tiles

# ---------------------------------------------------------------- waitfix ---
# walrus CoreV3 codegen allows a single sync-wait slot per instruction; Tile
# sometimes emits more. Split the tail drain and insert same-engine NoOps
# carrying the excess waits.
_counter = [0]


def _patch_tile_drain():
    from concourse.tile import ScopedClock

    if getattr(tile.TileContext, "_drain_patched", False):
        return

    def _drain_and_barrier(self, tick_clock, wait_clock):
        nc = self.nc
        drain_inst = nc.sync.drain()
        wait_clock.add_sem_waits(
            drain_inst.ins, ScopedClock({None: tick_clock.global_clock})
        )
        si = drain_inst.ins.sync_info
        waits = list(si.on_wait) if si and si.on_wait else []
        if len(waits) > 1:
            si.on_wait = waits[:1]
            drain_inst.ins.sync_info = si
            for w in waits[1:]:
                d = nc.sync.drain()
                d.ins.sync_info = mybir.SyncInfo(on_wait=[w], on_update=[])
        nc.all_engine_barrier()
        assert self.sems is not None
        popped = nc._tile_sem_poison_stack.pop()
        assert popped is self._sem_poison
        nc.clear_and_free_semaphores(list(self.sems.allocated().values()))
        nc.all_engine_barrier()

    tile.TileContext._drain_and_barrier = _drain_and_barrier
    tile.TileContext._drain_patched = True


def _fix_sync_waits(nc):
    for fn in nc.m.functions:
        for bb in fn.blocks:
            out = []
            changed = False
            for inst in bb.instructions:
                si = inst.sync_info
                waits = list(si.on_wait) if si and si.on_wait else []
                if len(waits) > 1:
                    for w in waits[1:]:
                        _counter[0] += 1
                        nop = mybir.InstNoOp(
                            name=f"I-wfix{_counter[0]}", ins=[], outs=[]
                        )
                        nop.engine = inst.engine
                        nop.sync_info = mybir.SyncInfo(on_wait=[w], on_update=[])
                        nc.register_instruction(nop)
                        out.append(nop)
                    si.on_wait = waits[:1]
                    inst.sync_info = si
                    changed = True
                out.append(inst)
            if changed:
                bb.instructions = out


# ------------------------------------------------------------- host RNN ----
def _host_rnn(x, embed, W1, W2, v, W_ih, b_ih, W_hh):
    """Exact fp32 mirror of the reference scan. Returns out [B,T,H], h_fin."""
    emb = embed[x]                                   # [B,T,E]
    h = np.zeros((B, H), np.float32)
    hist = np.zeros((B, T, H), np.float32)
    outs = np.empty((B, T, H), np.float32)
    tids = np.arange(T)
    W1T, W2T = W1.T.copy(), W2.T.copy()
    WihT, WhhT = W_ih.T.copy(), W_hh.T.copy()
    for t in range(T):
        qk = np.tanh((h @ W1T)[:, None, :] + hist @ W2T)     # [B,T,H]
        scores = qk @ v                                      # [B,T]
        scores = np.where(tids[None, :] < t, scores, NEG).astype(np.float32)
        m = scores.max(axis=1, keepdims=True)
        e = np.exp(scores - m)
        alpha = e / e.sum(axis=1, keepdims=True)
        context = np.einsum("bt,bth->bh", alpha, hist).astype(np.float32)
        rnn_in = np.concatenate([emb[:, t, :], context], axis=1)
        h = np.tanh(rnn_in @ WihT + b_ih + h @ WhhT).astype(np.float32)
        hist[:, t, :] = h
        outs[:, t, :] = h
    return outs, h


# --------------------------------------------------------- device kernel ---
_cache = {}


def _build_nc():
    if "nc" in _cache:
        return _cache["nc"]
    _patch_tile_drain()
    nc = bass.Bass("TRN2", target_bir_lowering=False, debug=False,
                   num_devices=N_CORES)
    f32, f32r = mybir.dt.float32, mybir.dt.float32r

    outT_d = nc.dram_tensor("outT", [128, 2 * M], f32, kind="ExternalInput").ap()
    fcw_d = nc.dram_tensor("fcwT", [128, 2, VLOC], f32, kind="ExternalInput").ap()
    logits_d = nc.dram_tensor("logits", [M, VLOC], f32, kind="ExternalOutput").ap()

    with tile.TileContext(nc) as tc:
        with (
            tc.tile_pool(name="stat", bufs=1) as stat,
            tc.tile_pool(name="wpool", bufs=3) as wpool,
            tc.tile_pool(name="psum", bufs=8, space="PSUM") as psum,
            tc.tile_pool(name="res", bufs=6) as rpool,
        ):
            # activations, resident: f32 staging + fp32r operand copy
            oT = stat.tile([128, 2 * M], f32)
            nc.sync.dma_start(oT[:], outT_d[:])
            oTr = stat.tile([128, 2 * M], f32r)
            # cast in 4 slices so the scheduler can overlap with weight DMAs
            for s in range(4):
                sl = bass.ts(s, 2 * M // 4)
                nc.vector.tensor_copy(oTr[:, sl], oT[:, sl])

            for n in range(NT):
                fw = wpool.tile([128, 2 * NN], f32, tag="fw")
                nc.sync.dma_start(
                    fw[:].rearrange("p (c n) -> p c n", c=2),
                    fcw_d[:, :, bass.ts(n, NN)],
                )
                fwr = wpool.tile([128, 2 * NN], f32r, tag="fwr")
                nc.vector.tensor_copy(fwr[:], fw[:])
                for m in range(MT):
                    ps = psum.tile([128, NN], f32)
                    nc.tensor.matmul(
                        ps[:], oTr[:, m * 128:(m + 1) * 128], fwr[:, 0:NN],
                        start=True, stop=False,
                    )
                    nc.tensor.matmul(
                        ps[:], oTr[:, M + m * 128:M + (m + 1) * 128],
                        fwr[:, NN:2 * NN], start=False, stop=True,
                    )
                    res = rpool.tile([128, NN], f32, tag="res")
                    # split PSUM->SBUF copies across DVE and ACT
                    if m % 2 == 0:
                        nc.vector.tensor_copy(res[:], ps[:])
                    else:
                        nc.scalar.copy(res[:], ps[:])
                    nc.sync.dma_start(
                        logits_d[m * 128:(m + 1) * 128, bass.ts(n, NN)], res[:]
                    )

    _fix_sync_waits(nc)
    _cache["nc"] = nc
    return nc


def kernel(x, embed, W1, W2, v, W_ih, b_ih, W_hh, fc_W, fc_b):
    x = np.asarray(x)
    embed = np.asarray(embed, np.float32)
    W1 = np.asarray(W1, np.float32)
    W2 = np.asarray(W2, np.float32)
    v = np.asarray(v, np.float32)
    W_ih = np.asarray(W_ih, np.float32)
    b_ih = np.asarray(b_ih, np.float32)
    W_hh = np.asarray(W_hh, np.float32)
    fc_W = np.asarray(fc_W, np.float32)
    fc_b = np.asarray(fc_b, np.float32)

    out, h_fin = _host_rnn(x, embed, W1, W2, v, W_ih, b_ih, W_hh)

    # outT[p, c*M + m] = out[m, c*128+p]   (m = b*T+t)
    o2 = out.reshape(M, H).T.reshape(2, 128, M)       # [c, p, m]
    outT = np.ascontiguousarray(o2.transpose(1, 0, 2).reshape(128, 2 * M))
    # fcwT[p, c, n] = fc_W[vbase+n, c*128+p]
    fcwT_full = fc_W.T.reshape(2, 128, V).transpose(1, 0, 2)  # [p, c, v]

    in_maps = []
    for c in range(N_CORES):
        in_maps.append({
            "outT": outT,
            "fcwT": np.ascontiguousarray(
                fcwT_full[:, :, c * VLOC:(c + 1) * VLOC]
            ),
        })

    nc = _build_nc()
    _cache["in_maps"] = in_maps
    res = bass_utils.run_bass_kernel_spmd(nc, in_maps, core_ids=list(range(N_CORES)))

    logits = np.empty((B, T, V), np.float32)
    for c in range(N_CORES):
        logits[:, :, c * VLOC:(c + 1) * VLOC] = (
            res.results[c]["logits"].reshape(B, T, VLOC)
        )
    if fc_b.any():
        logits += fc_b[None, None, :]
    return logits, h_fin


# revision 7
# speedup vs baseline: 1.0147x; 1.0147x over previous
"""AttentionRNN Trainium2 kernel.

Strategy (memory-regime): the output projection logits = out @ fc_W.T + fc_b
is 99.6% of the FLOPs and ~all of the memory traffic (524 MB of f32 logits).
It runs on the 8 NeuronCores, sharded over the vocab dimension (each core
reads its fc_W slice [4000, 256] and writes logits [32*128, 4000]) with
float32r matmuls (measured 1.5e-4 rel err on HW, 4x the fp32 PE rate).
The tiny strictly-sequential attention-RNN recurrence (0.13 GFLOP over 128
dependent steps) runs in exact fp32 on host.
"""
import numpy as np

import concourse.bass as bass
import concourse.mybir as mybir
import concourse.tile as tile
from concourse import bass_utils

B, T, V, E, H = 32, 128, 32000, 128, 256
NEG = -1e9
N_CORES = 8
VLOC = V // N_CORES            # 4000
NT = 8                         # V tiles per core
NN = VLOC // NT                # 500 columns per tile
M = B * T                      # 4096 rows
MT = M // 128                  # 32 row tiles

# ---------------------------------------------------------------- waitfix ---
# walrus CoreV3 codegen allows a single sync-wait slot per instruction; Tile
# sometimes emits more. Split the tail drain and insert same-engine NoOps
# carrying the excess waits.
_counter = [0]


def _patch_tile_drain():
    from concourse.tile import ScopedClock

    if getattr(tile.TileContext, "_drain_patched", False):
        return

    def _drain_and_barrier(self, tick_clock, wait_clock):
        nc = self.nc
        drain_inst = nc.sync.drain()
        wait_clock.add_sem_waits(
            drain_inst.ins, ScopedClock({None: tick_clock.global_clock})
        )
        si = drain_inst.ins.sync_info
        waits = list(si.on_wait) if si and si.on_wait else []
        if len(waits) > 1:
            si.on_wait = waits[:1]
            drain_inst.ins.sync_info = si
            for w in waits[1:]:
                d = nc.sync.drain()
                d.ins.sync_info = mybir.SyncInfo(on_wait=[w], on_update=[])
        nc.all_engine_barrier()
        assert self.sems is not None
        popped = nc._tile_sem_poison_stack.pop()
        assert popped is self._sem_poison
        nc.clear_and_free_semaphores(list(self.sems.allocated().values()))
        nc.all_engine_barrier()

    tile.TileContext._drain_and_barrier = _drain_and_barrier
    tile.TileContext._drain_patched = True


def _fix_sync_waits(nc):
    for fn in nc.m.functions:
        for bb in fn.blocks:
            out = []
            changed = False
            for inst in bb.instructions:
                si = inst.sync_info
                waits = list(si.on_wait) if si and si.on_wait else []
                if len(waits) > 1:
                    for w in waits[1:]:
                        _counter[0] += 1
                        nop = mybir.InstNoOp(
                            name=f"I-wfix{_counter[0]}", ins=[], outs=[]
                        )
                        nop.engine = inst.engine
                        nop.sync_info = mybir.SyncInfo(on_wait=[w], on_update=[])
                        nc.register_instruction(nop)
                        out.append(nop)
                    si.on_wait = waits[:1]
                    inst.sync_info = si
                    changed = True
                out.append(inst)
            if changed:
                bb.instructions = out


# ------------------------------------------------------------- host RNN ----
def _host_rnn(x, embed, W1, W2, v, W_ih, b_ih, W_hh):
    """Exact fp32 mirror of the reference scan. Returns out [B,T,H], h_fin."""
    emb = embed[x]                                   # [B,T,E]
    h = np.zeros((B, H), np.float32)
    hist = np.zeros((B, T, H), np.float32)
    outs = np.empty((B, T, H), np.float32)
    tids = np.arange(T)
    W1T, W2T = W1.T.copy(), W2.T.copy()
    WihT, WhhT = W_ih.T.copy(), W_hh.T.copy()
    for t in range(T):
        qk = np.tanh((h @ W1T)[:, None, :] + hist @ W2T)     # [B,T,H]
        scores = qk @ v                                      # [B,T]
        scores = np.where(tids[None, :] < t, scores, NEG).astype(np.float32)
        m = scores.max(axis=1, keepdims=True)
        e = np.exp(scores - m)
        alpha = e / e.sum(axis=1, keepdims=True)
        context = np.einsum("bt,bth->bh", alpha, hist).astype(np.float32)
        rnn_in = np.concatenate([emb[:, t, :], context], axis=1)
        h = np.tanh(rnn_in @ WihT + b_ih + h @ WhhT).astype(np.float32)
        hist[:, t, :] = h
        outs[:, t, :] = h
    return outs, h


# --------------------------------------------------------- device kernel ---
_cache = {}


def _build_nc():
    if "nc" in _cache:
        return _cache["nc"]
    _patch_tile_drain()
    nc = bass.Bass("TRN2", target_bir_lowering=False, debug=False,
                   num_devices=N_CORES)
    f32, f32r = mybir.dt.float32, mybir.dt.float32r

    outT_d = nc.dram_tensor("outT", [128, 2 * M], f32, kind="ExternalInput").ap()
    fcw_d = nc.dram_tensor("fcwT", [128, 2, VLOC], f32, kind="ExternalInput").ap()
    logits_d = nc.dram_tensor(
        "logits", [MT, NT, 128, NN], f32, kind="ExternalOutput"
    ).ap()  # tile-contiguous: each store is one linear 256 KB block

    with tile.TileContext(nc) as tc:
        with (
            tc.tile_pool(name="stat", bufs=1) as stat,
            tc.tile_pool(name="wpool", bufs=3) as wpool,
            tc.tile_pool(name="psum", bufs=8, space="PSUM") as psum,
            tc.tile_pool(name="res", bufs=6) as rpool,
        ):
            # activations, resident: f32 staging + fp32r operand copy.
            # DMA + cast in (chunk0, chunk1) column-slice pairs so the first
            # m-tiles' operands (needing both K-chunks) are ready early and
            # the first matmuls overlap the rest of the activation load.
            oT = stat.tile([128, 2 * M], f32)
            oTr = stat.tile([128, 2 * M], f32r)
            SS = M // 4
            for s in range(4):
                for c in range(2):
                    sl = slice(c * M + s * SS, c * M + (s + 1) * SS)
                    nc.sync.dma_start(oT[:, sl], outT_d[:, sl])
                    nc.vector.tensor_copy(oTr[:, sl], oT[:, sl])

            for n in range(NT):
                fw = wpool.tile([128, 2 * NN], f32, tag="fw")
                nc.sync.dma_start(
                    fw[:].rearrange("p (c n) -> p c n", c=2),
                    fcw_d[:, :, bass.ts(n, NN)],
                )
                fwr = wpool.tile([128, 2 * NN], f32r, tag="fwr")
                nc.vector.tensor_copy(fwr[:], fw[:])
                for m in range(MT):
                    ps = psum.tile([128, NN], f32)
                    nc.tensor.matmul(
                        ps[:], oTr[:, m * 128:(m + 1) * 128], fwr[:, 0:NN],
                        start=True, stop=False,
                    )
                    nc.tensor.matmul(
                        ps[:], oTr[:, M + m * 128:M + (m + 1) * 128],
                        fwr[:, NN:2 * NN], start=False, stop=True,
                    )
                    res = rpool.tile([128, NN], f32, tag="res")
                    # split PSUM->SBUF copies across DVE and ACT
                    if m % 2 == 0:
                        nc.vector.tensor_copy(res[:], ps[:])
                    else:
                        nc.scalar.copy(res[:], ps[:])
                    nc.sync.dma_start(logits_d[m, n], res[:])

    _fix_sync_waits(nc)
    _cache["nc"] = nc
    return nc


def kernel(x, embed, W1, W2, v, W_ih, b_ih, W_hh, fc_W, fc_b):
    x = np.asarray(x)
    embed = np.asarray(embed, np.float32)
    W1 = np.asarray(W1, np.float32)
    W2 = np.asarray(W2, np.float32)
    v = np.asarray(v, np.float32)
    W_ih = np.asarray(W_ih, np.float32)
    b_ih = np.asarray(b_ih, np.float32)
    W_hh = np.asarray(W_hh, np.float32)
    fc_W = np.asarray(fc_W, np.float32)
    fc_b = np.asarray(fc_b, np.float32)

    out, h_fin = _host_rnn(x, embed, W1, W2, v, W_ih, b_ih, W_hh)

    # outT[p, c*M + m] = out[m, c*128+p]   (m = b*T+t)
    o2 = out.reshape(M, H).T.reshape(2, 128, M)       # [c, p, m]
    outT = np.ascontiguousarray(o2.transpose(1, 0, 2).reshape(128, 2 * M))
    # fcwT[p, c, n] = fc_W[vbase+n, c*128+p]
    fcwT_full = fc_W.T.reshape(2, 128, V).transpose(1, 0, 2)  # [p, c, v]

    in_maps = []
    for c in range(N_CORES):
        in_maps.append({
            "outT": outT,
            "fcwT": np.ascontiguousarray(
                fcwT_full[:, :, c * VLOC:(c + 1) * VLOC]
            ),
        })

    nc = _build_nc()
    _cache["in_maps"] = in_maps
    res = bass_utils.run_bass_kernel_spmd(nc, in_maps, core_ids=list(range(N_CORES)))

    logits = np.empty((B, T, V), np.float32)
    for c in range(N_CORES):
        arr = res.results[c]["logits"].reshape(MT, NT, 128, NN)
        logits[:, :, c * VLOC:(c + 1) * VLOC] = (
            arr.transpose(0, 2, 1, 3).reshape(M, VLOC).reshape(B, T, VLOC)
        )
    if fc_b.any():
        logits += fc_b[None, None, :]
    return logits, h_fin
